# revision 1
# baseline (speedup 1.0000x reference)
"""Trainium2 Bass kernel for nn_ContrastiveMSELoss (8192x8192 cos-sim contrastive + MSE).

Sharding: 8 NeuronCores, users row-sharded 1024/core, full recipe table per core.

The loss decomposes so the 8192x8192 ratings matrix is never materialized:
    rowR[i]  = 0.1*M + sum_{final scatter cells in row i}(v - 0.1)
    S1       = 0.1*T + sum_pairs (v-0.1)*cos[u,i],  T = (sum_i u_i/|u_i|) . (sum_j r_j/|r_j|)
    S2       = sum_i rowR[i] * log(rowsum_exp[i])
    S3       = sum_i rowR[i] * log(colsum_exp[i])    (col_sum indexed by i: torch n==m quirk)
    loss     = 0.5*(S2 + S3 - 2*S1)/(2*N) + 0.5*mean((ratings-cos_sim)^2)

Per core: normalize R in a per-1024-column pipelined chain (square/reduce/ln/exp/mul ->
PE transpose -> bf16 [64, 8192]); cos tiles via PE matmul with 1/|u| folded into the ACT
exp's per-partition scale; row sums via a DVE tensor_scalar+accum pass over the bf16 exp
tiles; column sums via ones-matmul PSUM accumulation (two concurrent col-tiled matmuls);
per-block colsum partials DMA'd to DRAM progressively and ReduceScatter'd across cores;
scattered-pair cos via dma_gather of pre-normalized rows + DVE dots, scheduled in the
collective's shadow. Host does index prep (dedup last-write-wins, bincount, sharding)
and sums 8x5 partial scalars.
"""

import sys

sys.path.insert(0, "/opt/trn_rl_repo")

import numpy as np

import concourse.bass as bass
import concourse.bacc as bacc
import concourse.tile as tile
from concourse import mybir
from concourse.bass_utils import run_bass_kernel_spmd
from concourse.masks import make_identity

f32 = mybir.dt.float32
bf16 = mybir.dt.bfloat16
i16 = mybir.dt.int16
AF = mybir.ActivationFunctionType
OP = mybir.AluOpType
AX = mybir.AxisListType

NCORES = 8
N = 8192          # users
M = 8192          # recipes
D = 64
B = 65536
S = N // NCORES   # slab rows per core (1024)
RT = S // 128     # row tiles per slab (8)
NG = 8            # column groups of 1024
ALPHA = 0.5
FILL = 0.1
GATHER_CHUNK = 512  # descriptor-ring capacity limits idxs per dma_gather


def build_nc(K):
    """Build the SPMD Bass program. K = pairs per partition (128*K pair slots/core)."""
    nc = bacc.Bacc(num_devices=NCORES)

    u_slab = nc.declare_dram_parameter("u_slab", [S, D], f32, isOutput=False)
    r_full = nc.declare_dram_parameter("r_full", [M, D], f32, isOutput=False)
    row_r_slab = nc.declare_dram_parameter("row_r_slab", [S], f32, isOutput=False)
    pair_u = nc.declare_dram_parameter("pair_u", [128, 8 * K], i16, isOutput=False)
    pair_i = nc.declare_dram_parameter("pair_i", [128, 8 * K], i16, isOutput=False)
    pair_w = nc.declare_dram_parameter("pair_w", [128 * K], f32, isOutput=False)
    mse_ab = nc.declare_dram_parameter("mse_ab", [2 * (B // NCORES)], f32, isOutput=False)
    out_d = nc.declare_dram_parameter("out", [1, 8], f32, isOutput=True)

    NP = 128 * K

    with tile.TileContext(nc) as tc:
        with tc.tile_pool(name="sb", bufs=1) as sb, \
             tc.tile_pool(name="dram", bufs=1, space="DRAM") as dpool:
            # ---- constants ----
            ident = sb.tile([128, 128], f32)
            make_identity(nc, ident[:])
            ones_bf = sb.tile([128, 1], bf16)
            nc.vector.memset(ones_bf[:], 1.0)
            ones_f = sb.tile([128, 1], f32)
            nc.vector.memset(ones_f[:], 1.0)

            # ---- input loads ----
            u_raw = sb.tile([128, RT, D], f32)   # user r*128+p -> [p, r, :]
            nc.sync.dma_start(out=u_raw[:], in_=u_slab[:].rearrange("(r p) d -> p r d", p=128))
            r_raw = sb.tile([128, 64, D], f32)   # recipe n*128+p -> [p, n, :]
            nc.sync.dma_start(out=r_raw[:], in_=r_full[:].rearrange("(n p) d -> p n d", p=128))
            pu = sb.tile([128, NP // 16], i16)
            nc.sync.dma_start(out=pu[:], in_=pair_u[:])
            pi = sb.tile([128, NP // 16], i16)
            nc.sync.dma_start(out=pi[:], in_=pair_i[:])

            un_dram = dpool.tile([S, D], f32)
            rn_dram = dpool.tile([M, D], f32)
            cc_in = dpool.tile([M], f32)
            cc_out = dpool.tile([S], f32)

            with tc.tile_pool(name="psM", bufs=1, space="PSUM") as psM:
                # ---- U chain: invu + normalized copy + transpose ----
                usq = sb.tile([128, RT, D], f32)
                nc.vector.tensor_tensor(out=usq[:], in0=u_raw[:], in1=u_raw[:], op=OP.mult)
                ssq_u = sb.tile([128, RT], f32)
                nc.vector.tensor_reduce(out=ssq_u[:], in_=usq[:], axis=AX.X, op=OP.add)
                lssq_u = sb.tile([128, RT], f32)
                nc.scalar.activation(out=lssq_u[:], in_=ssq_u[:], func=AF.Ln)
                invu = sb.tile([128, RT], f32)
                nc.scalar.activation(out=invu[:], in_=lssq_u[:], func=AF.Exp, scale=-0.5)
                un = sb.tile([128, RT, D], f32)
                nc.vector.tensor_tensor(
                    out=un[:], in0=u_raw[:],
                    in1=invu[:, :, None].to_broadcast([128, RT, D]), op=OP.mult)
                nc.sync.dma_start(out=un_dram[:].rearrange("(r p) d -> p r d", p=128), in_=un[:])

                UT = sb.tile([64, S], bf16)
                ptu = psM.tile([64, 1024], f32, tag="tr", bufs=1)
                for r in range(RT):
                    nc.tensor.transpose(
                        out=ptu[:, r * 128:(r + 1) * 128], in_=u_raw[:, r, :], identity=ident[:])
                nc.vector.tensor_copy(out=UT[:], in_=ptu[:])

                # pair U gathers can start as soon as un_dram lands
                ug = sb.tile([128, K, D], f32)
                rg = sb.tile([128, K, D], f32)
                for off in range(0, NP, GATHER_CHUNK):
                    n = min(GATHER_CHUNK, NP - off)
                    nc.gpsimd.dma_gather(
                        ug[:, off // 128:(off + n) // 128, :], un_dram[:],
                        pu[:, off // 16:(off + n) // 16], n, n, D)

                # ---- R normalize pipeline (per column group g of 1024) ----
                RT_sb = sb.tile([64, M], bf16)
                sr_parts = sb.tile([64, NG], f32)
                ssq_r = sb.tile([128, 64], f32)
                invr = sb.tile([128, 64], f32)
                rhat = sb.tile([128, 64, D], f32)

                def phase_a(g):
                    gs = slice(g * 8, (g + 1) * 8)
                    rsq = sb.tile([128, 8, D], f32, tag="rsq", bufs=2)
                    nc.vector.tensor_tensor(
                        out=rsq[:], in0=r_raw[:, gs, :], in1=r_raw[:, gs, :], op=OP.mult)
                    nc.vector.tensor_reduce(
                        out=ssq_r[:, gs], in_=rsq[:], axis=AX.X, op=OP.add)
                    lss = sb.tile([128, 8], f32, tag="lss", bufs=2)
                    nc.scalar.activation(out=lss[:], in_=ssq_r[:, gs], func=AF.Ln)
                    nc.scalar.activation(out=invr[:, gs], in_=lss[:], func=AF.Exp, scale=-0.5)
                    nc.vector.tensor_tensor(
                        out=rhat[:, gs, :], in0=r_raw[:, gs, :],
                        in1=invr[:, gs][:, :, None].to_broadcast([128, 8, D]), op=OP.mult)
                    # write normalized rows to DRAM for the pair gathers
                    nc.sync.dma_start(
                        out=rn_dram[g * 1024:(g + 1) * 1024, :].rearrange("(n p) d -> p n d", p=128),
                        in_=rhat[:, gs, :])
                    ptr = psM.tile([64, 1024], f32, tag="tr", bufs=1)
                    for t in range(8):
                        nc.tensor.transpose(
                            out=ptr[:, t * 128:(t + 1) * 128], in_=rhat[:, g * 8 + t, :],
                            identity=ident[:])
                    nc.vector.tensor_scalar(
                        out=RT_sb[:, g * 1024:(g + 1) * 1024], in0=ptr[:],
                        scalar1=1.0, scalar2=None, op0=OP.mult, op1=OP.add,
                        accum_out=sr_parts[:, g:g + 1])

                # ---- main loop state ----
                rs_acc = sb.tile([128, RT * NG], f32)

                def phase_b(g):
                    cs_g = psM.tile([128, 512], f32, tag="cs", bufs=2, name=f"cs{g}")
                    for r in range(RT):
                        pg = psM.tile([128, 1024], f32, tag="cos", bufs=2)
                        for jj in range(2):
                            j = g * 2 + jj
                            nc.tensor.matmul(
                                out=pg[:, jj * 512:(jj + 1) * 512],
                                lhsT=UT[:, r * 128:(r + 1) * 128],
                                rhs=RT_sb[:, j * 512:(j + 1) * 512],
                                start=True, stop=True)
                        ex = sb.tile([128, 1024], bf16, tag="exp", bufs=4)
                        nc.scalar.activation(
                            out=ex[:], in_=pg[:], func=AF.Exp, scale=invu[:, r:r + 1])
                        tspo = sb.tile([128, 1024], bf16, tag="tsp", bufs=2)
                        nc.vector.tensor_scalar(
                            out=tspo[:], in0=ex[:], scalar1=1.0, scalar2=None,
                            op0=OP.mult, op1=OP.add,
                            accum_out=rs_acc[:, r * NG + g:r * NG + g + 1])
                        for jj in range(2):
                            nc.tensor.matmul(
                                out=cs_g[32 * jj:32 * jj + 1, :],
                                lhsT=ones_bf[:, 0:1],
                                rhs=ex[:, jj * 512:(jj + 1) * 512],
                                start=(r == 0), stop=(r == RT - 1),
                                tile_position=(0, 32 * jj),
                                skip_group_check=True)
                    # colsum partials for blocks 2g, 2g+1 -> DRAM (progressive)
                    csb = sb.tile([128, 2, 512], f32, tag="csb", bufs=2, name=f"csb{g}")
                    nc.vector.tensor_copy(out=csb[0:1, 0, :], in_=cs_g[0:1, :])
                    nc.vector.tensor_copy(out=csb[32:33, 1, :], in_=cs_g[32:33, :])
                    nc.sync.dma_start(out=cc_in[(2 * g) * 512:(2 * g + 1) * 512], in_=csb[0:1, 0, :])
                    nc.sync.dma_start(out=cc_in[(2 * g + 1) * 512:(2 * g + 2) * 512], in_=csb[32:33, 1, :])

                for g in range(NG):
                    phase_a(g)
                # rn_dram writes all issued; R-row gathers are ordered after them
                for off in range(0, NP, GATHER_CHUNK):
                    n = min(GATHER_CHUNK, NP - off)
                    nc.gpsimd.dma_gather(
                        rg[:, off // 128:(off + n) // 128, :], rn_dram[:],
                        pi[:, off // 16:(off + n) // 16], n, n, D)
                for g in range(NG):
                    phase_b(g)

            # =============== collective: ReduceScatter column sums ===============
            nc.gpsimd.collective_compute(
                "ReduceScatter", OP.add,
                replica_groups=[list(range(NCORES))],
                ins=[cc_in[:].opt()], outs=[cc_out[:].opt()])

            # =============== tail (overlaps the collective) ===============
            with tc.tile_pool(name="psT", bufs=1, space="PSUM") as psT:
                # T partial: sum_{p,r} invu * (u_raw . sR)
                sr_f = sb.tile([64, 1], f32)
                nc.vector.tensor_reduce(out=sr_f[:], in_=sr_parts[:], axis=AX.X, op=OP.add)
                sr_bf = sb.tile([64, 1], bf16)
                nc.vector.tensor_copy(out=sr_bf[:], in_=sr_f[:])
                psTT = psT.tile([128, RT], f32)
                for r in range(RT):
                    nc.tensor.matmul(
                        out=psTT[:, r:r + 1], lhsT=UT[:, r * 128:(r + 1) * 128],
                        rhs=sr_bf[:], start=True, stop=True)
                tdot = sb.tile([128, RT], f32)
                nc.vector.tensor_copy(out=tdot[:], in_=psTT[:])
                tw = sb.tile([128, RT], f32)
                nc.vector.tensor_tensor(out=tw[:], in0=tdot[:], in1=invu[:], op=OP.mult)
                t_acc = sb.tile([128, 1], f32)
                nc.vector.tensor_reduce(out=t_acc[:], in_=tw[:], axis=AX.X, op=OP.add)

                # pair term: cos = un[u] . rn[i] (rows pre-normalized)
                prod = sb.tile([128, K, D], f32)
                nc.vector.tensor_tensor(out=prod[:], in0=ug[:], in1=rg[:], op=OP.mult)
                cosg = sb.tile([128, K], f32)
                nc.vector.tensor_reduce(out=cosg[:], in_=prod[:], axis=AX.X, op=OP.add)
                pw = sb.tile([128, K], f32)
                nc.sync.dma_start(out=pw[:], in_=pair_w[:].rearrange("(c p) -> p c", p=128))
                cosgw = sb.tile([128, K], f32)
                nc.vector.tensor_tensor(out=cosgw[:], in0=cosg[:], in1=pw[:], op=OP.mult)
                w_acc = sb.tile([128, 1], f32)
                nc.vector.tensor_reduce(out=w_acc[:], in_=cosgw[:], axis=AX.X, op=OP.add)

                # S2: sum rowR_slab * ln(rowsum)
                rs_r = sb.tile([128, RT], f32)
                nc.vector.tensor_reduce(
                    out=rs_r[:], in_=rs_acc[:].rearrange("p (r g) -> p r g", g=NG),
                    axis=AX.X, op=OP.add)
                lrs = sb.tile([128, RT], f32)
                nc.scalar.activation(out=lrs[:], in_=rs_r[:], func=AF.Ln)
                rowr_sb = sb.tile([128, RT], f32)
                nc.sync.dma_start(out=rowr_sb[:], in_=row_r_slab[:].rearrange("(r p) -> p r", p=128))
                s2w = sb.tile([128, RT], f32)
                nc.vector.tensor_tensor(out=s2w[:], in0=lrs[:], in1=rowr_sb[:], op=OP.mult)
                s2_acc = sb.tile([128, 1], f32)
                nc.vector.tensor_reduce(out=s2_acc[:], in_=s2w[:], axis=AX.X, op=OP.add)

                # MSE
                mab = sb.tile([128, 128], f32)
                nc.sync.dma_start(out=mab[:], in_=mse_ab[:].rearrange("(p k) -> p k", p=128))
                md = sb.tile([128, 64], f32)
                nc.vector.tensor_tensor(out=md[:], in0=mab[:, 0:64], in1=mab[:, 64:128], op=OP.subtract)
                msq = sb.tile([128, 64], f32)
                nc.vector.tensor_tensor(out=msq[:], in0=md[:], in1=md[:], op=OP.mult)
                m_acc = sb.tile([128, 1], f32)
                nc.vector.tensor_reduce(out=m_acc[:], in_=msq[:], axis=AX.X, op=OP.add)

                # S3 (after ReduceScatter lands): sum rowR_slab * ln(colsum_slab)
                lcs_in = sb.tile([128, RT], f32)
                nc.sync.dma_start(out=lcs_in[:], in_=cc_out[:].rearrange("(r p) -> p r", p=128))
                lcs = sb.tile([128, RT], f32)
                nc.scalar.activation(out=lcs[:], in_=lcs_in[:], func=AF.Ln)
                s3w = sb.tile([128, RT], f32)
                nc.vector.tensor_tensor(out=s3w[:], in0=lcs[:], in1=rowr_sb[:], op=OP.mult)
                s3_acc = sb.tile([128, 1], f32)
                nc.vector.tensor_reduce(out=s3_acc[:], in_=s3w[:], axis=AX.X, op=OP.add)

                # partition-reduce the five partials via ones-matmul
                combo = sb.tile([128, 5], f32)
                nc.vector.tensor_copy(out=combo[:, 0:1], in_=s2_acc[:])
                nc.vector.tensor_copy(out=combo[:, 1:2], in_=s3_acc[:])
                nc.vector.tensor_copy(out=combo[:, 2:3], in_=t_acc[:])
                nc.vector.tensor_copy(out=combo[:, 3:4], in_=w_acc[:])
                nc.vector.tensor_copy(out=combo[:, 4:5], in_=m_acc[:])
                po = psT.tile([1, 5], f32)
                nc.tensor.matmul(out=po[:], lhsT=ones_f[:, 0:1], rhs=combo[:], start=True, stop=True)
                out_sb = sb.tile([1, 8], f32)
                nc.vector.memset(out_sb[:], 0.0)
                nc.vector.tensor_copy(out=out_sb[:, 0:5], in_=po[:])
                nc.sync.dma_start(out=out_d[:], in_=out_sb[:])
    nc.finalize()
    return nc


def _host_prep(inputs):
    """Dedup scatter (last write wins), shard pairs by row slab, build per-core arrays."""
    U = np.ascontiguousarray(np.asarray(inputs["user_embeddings"], dtype=np.float32))
    R = np.ascontiguousarray(np.asarray(inputs["recipe_embeddings"], dtype=np.float32))
    rat = np.asarray(inputs["ratings_scaled"], dtype=np.float32)
    css = np.asarray(inputs["cos_similarities_scaled"], dtype=np.float32)
    u = np.asarray(inputs["u_idx"]).astype(np.int64)
    i = np.asarray(inputs["i_idx"]).astype(np.int64)

    cell = u * M + i
    _, idx_rev = np.unique(cell[::-1], return_index=True)
    keep = (B - 1 - idx_rev)  # last occurrences, sorted by cell (=> sorted by u)
    uu = u[keep].astype(np.int32)
    ii = i[keep].astype(np.int32)
    ww = (rat[keep] - FILL).astype(np.float32)

    delta = np.bincount(uu, weights=ww.astype(np.float64), minlength=N)
    row_r = (FILL * M + delta).astype(np.float32)

    core_of = uu // S
    counts = np.bincount(core_of, minlength=NCORES)
    K = int(np.ceil(counts.max() / 128))
    cap = 128 * K

    in_maps = []
    bs = B // NCORES
    for c in range(NCORES):
        sel = core_of == c
        n_c = int(sel.sum())
        pu = np.zeros(cap, dtype=np.int16)
        pi = np.zeros(cap, dtype=np.int16)
        pw = np.zeros(cap, dtype=np.float32)
        pu[:n_c] = uu[sel] - c * S
        pi[:n_c] = ii[sel]
        pw[:n_c] = ww[sel]
        # dma_gather idx layout: [128, cap//16], row p = idx[s*16 + p%16], replicated 8x
        pu_dev = np.ascontiguousarray(np.tile(pu.reshape(cap // 16, 16).T, (8, 1)).astype(np.int16))
        pi_dev = np.ascontiguousarray(np.tile(pi.reshape(cap // 16, 16).T, (8, 1)).astype(np.int16))
        in_maps.append({
            "u_slab": np.ascontiguousarray(U[c * S:(c + 1) * S]),
            "r_full": R,
            "row_r_slab": np.ascontiguousarray(row_r[c * S:(c + 1) * S]),
            "pair_u": pu_dev,
            "pair_i": pi_dev,
            "pair_w": pw,
            "mse_ab": np.concatenate([
                rat[c * bs:(c + 1) * bs].reshape(128, 64),
                css[c * bs:(c + 1) * bs].reshape(128, 64)], axis=1).ravel(),
        })
    return in_maps, K


def kernel(user_embeddings, recipe_embeddings, ratings_scaled, cos_similarities_scaled,
           u_idx, i_idx, _trace=False):
    inputs = {
        "user_embeddings": user_embeddings,
        "recipe_embeddings": recipe_embeddings,
        "ratings_scaled": ratings_scaled,
        "cos_similarities_scaled": cos_similarities_scaled,
        "u_idx": u_idx,
        "i_idx": i_idx,
    }
    in_maps, K = _host_prep(inputs)
    nc = build_nc(K)
    res = run_bass_kernel_spmd(nc, in_maps, core_ids=list(range(NCORES)), trace=_trace)
    outs = np.stack([res.results[c]["out"][0] for c in range(NCORES)])  # [8, 8]
    o = outs.astype(np.float64)
    S2 = o[:, 0].sum()
    S3 = o[:, 1].sum()
    T = o[:, 2].sum()
    PAIR = o[:, 3].sum()
    MSE_SUM = o[:, 4].sum()
    contrastive = (S2 + S3 - 2.0 * (FILL * T + PAIR)) / (2.0 * N)
    loss = ALPHA * contrastive + (1.0 - ALPHA) * (MSE_SUM / B)
    if _trace:
        kernel._last_results = res
    return np.float32(loss)



# revision 4
# speedup vs baseline: 1.5634x; 1.5634x over previous
"""Trainium2 Bass kernel for nn_ContrastiveMSELoss (8192x8192 cos-sim contrastive + MSE).

Sharding: 8 NeuronCores, users row-sharded 1024/core, full recipe table per core.

The loss decomposes so the 8192x8192 ratings matrix is never materialized:
    rowR[i]  = 0.1*M + sum_{final scatter cells in row i}(v - 0.1)
    S1       = 0.1*T + sum_pairs (v-0.1)*cos[u,i],  T = (sum_i u_i/|u_i|) . (sum_j r_j/|r_j|)
    S2       = sum_i rowR[i] * log(rowsum_exp[i])
    S3       = sum_i rowR[i] * log(colsum_exp[i])    (col_sum indexed by i: torch n==m quirk)
    loss     = 0.5*(S2 + S3 - 2*S1)/(2*N) + 0.5*mean((ratings-cos_sim)^2)

v2: pair rows are host-gathered (bf16, u||r) instead of device dma_gather; the
row softmax sum rides the EXP activation's accum_out; ACT table loads batched
(Ln then Exp once each, tail Lns together); R transposed in bf16; column-sum
PSUM accumulators DMA straight to DRAM for the ReduceScatter. Host does index
prep (dedup last-write-wins, bincount, sharding) and sums 8x5 partial scalars.
"""

import sys

sys.path.insert(0, "/opt/trn_rl_repo")

import numpy as np

import concourse.bass as bass
import concourse.bacc as bacc
import concourse.tile as tile
from concourse import mybir
from concourse.bass_utils import run_bass_kernel_spmd
from concourse.masks import make_identity

f32 = mybir.dt.float32
bf16 = mybir.dt.bfloat16
AF = mybir.ActivationFunctionType
OP = mybir.AluOpType
AX = mybir.AxisListType

NCORES = 8
N = 8192          # users
M = 8192          # recipes
D = 64
B = 65536
S = N // NCORES   # slab rows per core (1024)
RT = S // 128     # row tiles per slab (8)
NG = 8            # column groups of 1024
ALPHA = 0.5
FILL = 0.1


def build_nc(K):
    """Build the SPMD Bass program. K = pairs per partition (128*K pair slots/core)."""
    nc = bacc.Bacc(num_devices=NCORES)

    u_slab = nc.declare_dram_parameter("u_slab", [S, D], f32, isOutput=False)
    r_full = nc.declare_dram_parameter("r_full", [M, D], f32, isOutput=False)
    row_r_slab = nc.declare_dram_parameter("row_r_slab", [S], f32, isOutput=False)
    pairs_d = nc.declare_dram_parameter("pairs", [128, K * 128], bf16, isOutput=False)
    pair_w = nc.declare_dram_parameter("pair_w", [128, K], f32, isOutput=False)
    mse_ab = nc.declare_dram_parameter("mse_ab", [2 * (B // NCORES)], f32, isOutput=False)
    out_d = nc.declare_dram_parameter("out", [1, 8], f32, isOutput=True)

    with tile.TileContext(nc) as tc:
        with tc.tile_pool(name="sb", bufs=1) as sb, \
             tc.tile_pool(name="dram", bufs=1, space="DRAM") as dpool:
            # ---- constants ----
            ident_bf = sb.tile([128, 128], bf16)
            make_identity(nc, ident_bf[:])
            ones_bf = sb.tile([128, 1], bf16)
            nc.vector.memset(ones_bf[:], 1.0)
            ones_f = sb.tile([128, 1], f32)
            nc.vector.memset(ones_f[:], 1.0)

            # ---- input loads ----
            u_raw = sb.tile([128, RT, D], f32)   # user r*128+p -> [p, r, :]
            nc.sync.dma_start(out=u_raw[:], in_=u_slab[:].rearrange("(r p) d -> p r d", p=128))
            r_raw = sb.tile([128, 64, D], f32)   # recipe n*128+p -> [p, n, :]
            nc.sync.dma_start(out=r_raw[:], in_=r_full[:].rearrange("(n p) d -> p n d", p=128))
            Pg = sb.tile([128, K, 128], bf16)    # pair slot (p,k): [0:64]=U row, [64:128]=R row
            nc.sync.dma_start(out=Pg[:], in_=pairs_d[:].rearrange("p (k d) -> p k d", d=128))
            pw = sb.tile([128, K], f32)
            nc.sync.dma_start(out=pw[:], in_=pair_w[:])
            rowr_sb = sb.tile([128, RT], f32)
            nc.sync.dma_start(out=rowr_sb[:], in_=row_r_slab[:].rearrange("(r p) -> p r", p=128))
            mab = sb.tile([128, 128], f32)
            nc.sync.dma_start(out=mab[:], in_=mse_ab[:].rearrange("(p k) -> p k", p=128))

            cc_in = dpool.tile([M], f32)
            cc_out = dpool.tile([S], f32)

            with tc.tile_pool(name="psM", bufs=1, space="PSUM") as psM:
                # ---- norms: squares + reduce (DVE), then batched Ln / Exp ----
                usq = sb.tile([128, RT, D], f32)
                nc.vector.tensor_tensor(out=usq[:], in0=u_raw[:], in1=u_raw[:], op=OP.mult)
                ssq_u = sb.tile([128, RT], f32)
                nc.vector.tensor_reduce(out=ssq_u[:], in_=usq[:], axis=AX.X, op=OP.add)
                rsq = sb.tile([128, 64, D], f32)
                nc.vector.tensor_tensor(out=rsq[:], in0=r_raw[:], in1=r_raw[:], op=OP.mult)
                ssq_r = sb.tile([128, 64], f32)
                nc.vector.tensor_reduce(out=ssq_r[:], in_=rsq[:], axis=AX.X, op=OP.add)

                lssq_u = sb.tile([128, RT], f32)
                nc.scalar.activation(out=lssq_u[:], in_=ssq_u[:], func=AF.Ln)
                lssq_r = sb.tile([128, 64], f32)
                nc.scalar.activation(out=lssq_r[:], in_=ssq_r[:], func=AF.Ln)
                invu = sb.tile([128, RT], f32)
                nc.scalar.activation(out=invu[:], in_=lssq_u[:], func=AF.Exp, scale=-0.5)
                invr = sb.tile([128, 64], f32)
                nc.scalar.activation(out=invr[:], in_=lssq_r[:], func=AF.Exp, scale=-0.5)

                # ---- normalized R (bf16) + raw U (bf16) for the PE transposes ----
                rhat = sb.tile([128, 64, D], bf16)
                nc.vector.tensor_tensor(
                    out=rhat[:], in0=r_raw[:],
                    in1=invr[:, :, None].to_broadcast([128, 64, D]), op=OP.mult)
                u_bf = sb.tile([128, RT, D], bf16)
                nc.vector.tensor_copy(out=u_bf[:], in_=u_raw[:])

                # ---- transposes: U then R per group, shared psum tag ----
                UT = sb.tile([64, S], bf16)
                ptu = psM.tile([64, 1024], bf16, tag="tr", bufs=2)
                for r in range(RT):
                    nc.tensor.transpose(
                        out=ptu[:, r * 128:(r + 1) * 128], in_=u_bf[:, r, :],
                        identity=ident_bf[:])
                nc.vector.tensor_copy(out=UT[:], in_=ptu[:])

                RT_sb = sb.tile([64, M], bf16)
                sr_parts = sb.tile([64, NG], f32)

                def phase_a(g):
                    ptr = psM.tile([64, 1024], bf16, tag="tr", bufs=2)
                    for t in range(8):
                        nc.tensor.transpose(
                            out=ptr[:, t * 128:(t + 1) * 128], in_=rhat[:, g * 8 + t, :],
                            identity=ident_bf[:])
                    nc.vector.tensor_scalar(
                        out=RT_sb[:, g * 1024:(g + 1) * 1024], in0=ptr[:],
                        scalar1=1.0, scalar2=None, op0=OP.mult, op1=OP.add,
                        accum_out=sr_parts[:, g:g + 1])

                # ---- main loop state ----
                rs_parts = sb.tile([128, RT * NG], f32)

                def phase_b(g):
                    cs_g = psM.tile([128, 512], f32, tag="cs", bufs=2, name=f"cs{g}")
                    for r in range(RT):
                        pg = psM.tile([128, 1024], f32, tag="cos", bufs=2)
                        for jj in range(2):
                            j = g * 2 + jj
                            nc.tensor.matmul(
                                out=pg[:, jj * 512:(jj + 1) * 512],
                                lhsT=UT[:, r * 128:(r + 1) * 128],
                                rhs=RT_sb[:, j * 512:(j + 1) * 512],
                                start=True, stop=True)
                        ex = sb.tile([128, 1024], bf16, tag="exp", bufs=4)
                        nc.scalar.activation(
                            out=ex[:], in_=pg[:], func=AF.Exp, scale=invu[:, r:r + 1],
                            accum_out=rs_parts[:, r * NG + g:r * NG + g + 1])
                        for jj in range(2):
                            nc.tensor.matmul(
                                out=cs_g[32 * jj:32 * jj + 1, :],
                                lhsT=ones_bf[:, 0:1],
                                rhs=ex[:, jj * 512:(jj + 1) * 512],
                                start=(r == 0), stop=(r == RT - 1),
                                tile_position=(0, 32 * jj),
                                skip_group_check=True)
                    # colsum partials for blocks 2g, 2g+1 -> DRAM (progressive);
                    # PSUM->SBUF bounce (GpSimd cannot read PSUM, DVE is idle here)
                    csb = sb.tile([128, 2, 512], f32, tag="csb", bufs=2, name=f"csb{g}")
                    nc.vector.tensor_copy(out=csb[0:1, 0, :], in_=cs_g[0:1, :])
                    nc.vector.tensor_copy(out=csb[32:33, 1, :], in_=cs_g[32:33, :])
                    nc.sync.dma_start(out=cc_in[(2 * g) * 512:(2 * g + 1) * 512], in_=csb[0:1, 0, :])
                    nc.sync.dma_start(out=cc_in[(2 * g + 1) * 512:(2 * g + 2) * 512], in_=csb[32:33, 1, :])

                for g in range(NG):
                    phase_a(g)

                # ---- pair math on DVE: overlaps phase_b (issue before, deps allow) ----
                sq = sb.tile([128, K, 128], bf16)
                nc.vector.tensor_tensor(out=sq[:], in0=Pg[:], in1=Pg[:], op=OP.mult)
                nrm2 = sb.tile([128, 2 * K], f32)
                nc.vector.tensor_reduce(
                    out=nrm2[:], in_=sq[:].rearrange("p k (h d) -> p (k h) d", h=2),
                    axis=AX.X, op=OP.add)
                n2 = sb.tile([128, K], f32)
                nc.vector.tensor_reduce(
                    out=n2[:], in_=nrm2[:].rearrange("p (k h) -> p k h", h=2),
                    axis=AX.X, op=OP.mult)
                prod = sb.tile([128, K, D], bf16)
                nc.vector.tensor_tensor(
                    out=prod[:], in0=Pg[:, :, 0:64], in1=Pg[:, :, 64:128], op=OP.mult)
                dot = sb.tile([128, K], f32)
                nc.vector.tensor_reduce(out=dot[:], in_=prod[:], axis=AX.X, op=OP.add)

                for g in range(NG):
                    phase_b(g)

            # =============== collective: ReduceScatter column sums ===============
            nc.gpsimd.collective_compute(
                "ReduceScatter", OP.add,
                replica_groups=[list(range(NCORES))],
                ins=[cc_in[:].opt()], outs=[cc_out[:].opt()])

            # =============== tail (overlaps the collective) ===============
            with tc.tile_pool(name="psT", bufs=1, space="PSUM") as psT:
                # T partial: sum_{p,r} invu * (u_raw . sR)
                sr_f = sb.tile([64, 1], f32)
                nc.vector.tensor_reduce(out=sr_f[:], in_=sr_parts[:], axis=AX.X, op=OP.add)
                sr_bf = sb.tile([64, 1], bf16)
                nc.vector.tensor_copy(out=sr_bf[:], in_=sr_f[:])
                psTT = psT.tile([128, RT], f32)
                for r in range(RT):
                    nc.tensor.matmul(
                        out=psTT[:, r:r + 1], lhsT=UT[:, r * 128:(r + 1) * 128],
                        rhs=sr_bf[:], start=True, stop=True)
                tdot = sb.tile([128, RT], f32)
                nc.vector.tensor_copy(out=tdot[:], in_=psTT[:])
                tw = sb.tile([128, RT], f32)
                nc.vector.tensor_tensor(out=tw[:], in0=tdot[:], in1=invu[:], op=OP.mult)
                t_acc = sb.tile([128, 1], f32)
                nc.vector.tensor_reduce(out=t_acc[:], in_=tw[:], axis=AX.X, op=OP.add)

                # S2: sum rowR_slab * ln(rowsum)   [ACT: Ln batch starts here]
                rs_r = sb.tile([128, RT], f32)
                nc.vector.tensor_reduce(
                    out=rs_r[:], in_=rs_parts[:].rearrange("p (r g) -> p r g", g=NG),
                    axis=AX.X, op=OP.add)
                lrs = sb.tile([128, RT], f32)
                nc.scalar.activation(out=lrs[:], in_=rs_r[:], func=AF.Ln)
                s2w = sb.tile([128, RT], f32)
                nc.vector.tensor_tensor(out=s2w[:], in0=lrs[:], in1=rowr_sb[:], op=OP.mult)
                s2_acc = sb.tile([128, 1], f32)
                nc.vector.tensor_reduce(out=s2_acc[:], in_=s2w[:], axis=AX.X, op=OP.add)

                # pair cos: ln(n2) -> later exp(-0.5*ln) after the other Lns
                ln2 = sb.tile([128, K], f32)
                nc.scalar.activation(out=ln2[:], in_=n2[:], func=AF.Ln)

                # S3 (after ReduceScatter lands): sum rowR_slab * ln(colsum_slab)
                lcs_in = sb.tile([128, RT], f32)
                nc.sync.dma_start(out=lcs_in[:], in_=cc_out[:].rearrange("(r p) -> p r", p=128))
                lcs = sb.tile([128, RT], f32)
                nc.scalar.activation(out=lcs[:], in_=lcs_in[:], func=AF.Ln)
                s3w = sb.tile([128, RT], f32)
                nc.vector.tensor_tensor(out=s3w[:], in0=lcs[:], in1=rowr_sb[:], op=OP.mult)
                s3_acc = sb.tile([128, 1], f32)
                nc.vector.tensor_reduce(out=s3_acc[:], in_=s3w[:], axis=AX.X, op=OP.add)

                # pair term: cos = dot * rsqrt(u2*r2), weighted by w
                inv = sb.tile([128, K], f32)
                nc.scalar.activation(out=inv[:], in_=ln2[:], func=AF.Exp, scale=-0.5)
                cosg = sb.tile([128, K], f32)
                nc.vector.tensor_tensor(out=cosg[:], in0=dot[:], in1=inv[:], op=OP.mult)
                cosgw = sb.tile([128, K], f32)
                nc.vector.tensor_tensor(out=cosgw[:], in0=cosg[:], in1=pw[:], op=OP.mult)
                w_acc = sb.tile([128, 1], f32)
                nc.vector.tensor_reduce(out=w_acc[:], in_=cosgw[:], axis=AX.X, op=OP.add)

                # MSE
                md = sb.tile([128, 64], f32)
                nc.vector.tensor_tensor(out=md[:], in0=mab[:, 0:64], in1=mab[:, 64:128], op=OP.subtract)
                msq = sb.tile([128, 64], f32)
                nc.vector.tensor_tensor(out=msq[:], in0=md[:], in1=md[:], op=OP.mult)
                m_acc = sb.tile([128, 1], f32)
                nc.vector.tensor_reduce(out=m_acc[:], in_=msq[:], axis=AX.X, op=OP.add)

                # partition-reduce the five partials via ones-matmul
                combo = sb.tile([128, 5], f32)
                nc.vector.tensor_copy(out=combo[:, 0:1], in_=s2_acc[:])
                nc.vector.tensor_copy(out=combo[:, 1:2], in_=s3_acc[:])
                nc.vector.tensor_copy(out=combo[:, 2:3], in_=t_acc[:])
                nc.vector.tensor_copy(out=combo[:, 3:4], in_=w_acc[:])
                nc.vector.tensor_copy(out=combo[:, 4:5], in_=m_acc[:])
                po = psT.tile([1, 5], f32)
                nc.tensor.matmul(out=po[:], lhsT=ones_f[:, 0:1], rhs=combo[:], start=True, stop=True)
                out_sb = sb.tile([1, 8], f32)
                nc.vector.memset(out_sb[:], 0.0)
                nc.vector.tensor_copy(out=out_sb[:, 0:5], in_=po[:])
                nc.sync.dma_start(out=out_d[:], in_=out_sb[:])
    nc.finalize()
    return nc


def _host_prep(inputs):
    """Dedup scatter (last write wins), shard pairs by row slab, build per-core arrays."""
    U = np.ascontiguousarray(np.asarray(inputs["user_embeddings"], dtype=np.float32))
    R = np.ascontiguousarray(np.asarray(inputs["recipe_embeddings"], dtype=np.float32))
    rat = np.asarray(inputs["ratings_scaled"], dtype=np.float32)
    css = np.asarray(inputs["cos_similarities_scaled"], dtype=np.float32)
    u = np.asarray(inputs["u_idx"]).astype(np.int64)
    i = np.asarray(inputs["i_idx"]).astype(np.int64)

    cell = u * M + i
    _, idx_rev = np.unique(cell[::-1], return_index=True)
    keep = (B - 1 - idx_rev)  # last occurrences, sorted by cell (=> sorted by u)
    uu = u[keep].astype(np.int32)
    ii = i[keep].astype(np.int32)
    ww = (rat[keep] - FILL).astype(np.float32)

    delta = np.bincount(uu, weights=ww.astype(np.float64), minlength=N)
    row_r = (FILL * M + delta).astype(np.float32)

    core_of = uu // S
    counts = np.bincount(core_of, minlength=NCORES)
    K = int(np.ceil(counts.max() / 128))
    cap = 128 * K

    import ml_dtypes
    Ub = U.astype(ml_dtypes.bfloat16)
    Rb = R.astype(ml_dtypes.bfloat16)

    in_maps = []
    bs = B // NCORES
    for c in range(NCORES):
        sel = core_of == c
        n_c = int(sel.sum())
        # pair buffer [128, K, 128]: slot (p, k) = pair k*128+p; [0:64]=U row, [64:128]=R row
        P = np.zeros((cap, 128), dtype=ml_dtypes.bfloat16)
        P[:, 0] = 1.0  # padding rows: unit basis vector (norm 1, cos 1, w 0)
        P[:n_c, 0:64] = Ub[uu[sel]]
        P[:n_c, 64:128] = Rb[ii[sel]]
        P = np.ascontiguousarray(
            P.reshape(K, 128, 128).transpose(1, 0, 2).reshape(128, K * 128))
        W = np.zeros((cap,), dtype=np.float32)
        W[:n_c] = ww[sel]
        W = np.ascontiguousarray(W.reshape(K, 128).T)
        in_maps.append({
            "u_slab": np.ascontiguousarray(U[c * S:(c + 1) * S]),
            "r_full": R,
            "row_r_slab": np.ascontiguousarray(row_r[c * S:(c + 1) * S]),
            "pairs": P,
            "pair_w": W,
            "mse_ab": np.concatenate([
                rat[c * bs:(c + 1) * bs].reshape(128, 64),
                css[c * bs:(c + 1) * bs].reshape(128, 64)], axis=1).ravel(),
        })
    return in_maps, K


def kernel(user_embeddings, recipe_embeddings, ratings_scaled, cos_similarities_scaled,
           u_idx, i_idx, _trace=False):
    inputs = {
        "user_embeddings": user_embeddings,
        "recipe_embeddings": recipe_embeddings,
        "ratings_scaled": ratings_scaled,
        "cos_similarities_scaled": cos_similarities_scaled,
        "u_idx": u_idx,
        "i_idx": i_idx,
    }
    in_maps, K = _host_prep(inputs)
    nc = build_nc(K)
    res = run_bass_kernel_spmd(nc, in_maps, core_ids=list(range(NCORES)), trace=_trace)
    outs = np.stack([res.results[c]["out"][0] for c in range(NCORES)])  # [8, 8]
    o = outs.astype(np.float64)
    S2 = o[:, 0].sum()
    S3 = o[:, 1].sum()
    T = o[:, 2].sum()
    PAIR = o[:, 3].sum()
    MSE_SUM = o[:, 4].sum()
    contrastive = (S2 + S3 - 2.0 * (FILL * T + PAIR)) / (2.0 * N)
    loss = ALPHA * contrastive + (1.0 - ALPHA) * (MSE_SUM / B)
    if _trace:
        kernel._last_results = res
    return np.float32(loss)
